# revision 9
# baseline (speedup 1.0000x reference)
"""Trainium2 Bass kernel v2: 4096x4096 fp32 image, 16x16 valid cross-corr + bias.

Block-output scheme: each psum element holds one output of a 16x8 block,
  psum[m=(a,b), n] = out[16s + a, 8n + b],
and the contraction dim holds a 16x8 patch of X,
  D_t[p=(rho,gamma), n] = X[16t + rho, 8n + gamma]
(i.e. X reshaped so partition = (row%16)*8 + col%8, free = col//8).
Each kernel tap (r, c) decomposes uniquely as r = rho - a + 16u (u in {0,1}),
c = gamma - b + 8v (v in {0,1,2}), so SIX matmuls accumulate the full 16x16
conv for 16 output rows x 4096 cols at once:
  psum += S_uv.T @ D_{s+u}[:, v:v+512],
  S_uv[p, m] = W[rho - a + 16u, gamma - b + 8v] (where in range, else 0).
This streams 6x512 PE columns per 16 output rows (vs 10x512 per 17 rows
for the banded-phase scheme) and loads each X byte exactly once (strips
stride = strip height; no halo duplication in HBM traffic).

Per strip: one 6-matmul chain into a single PSUM bank, then one DVE
tensor_scalar_add (+bias, bf16 cast) to SBUF, then SWDGE store. All
storage bf16 (fp32 PSUM accumulation); rel err vs fp32 reference ~3e-3.
Output rows sharded across 8 cores (512 rows each, 32 strips of 16);
weights and bias replicated.

Env (bench only): CONV_LOOP wraps the body in a hardware For_i loop.
"""
import os

import numpy as np

import concourse.mybir as mybir
import concourse.tile as tile
from concourse import bacc
from concourse.bass_utils import run_bass_kernel_spmd

H = 4096
W = 4096
KH = 16
KW = 16
OH = H - KH + 1  # 4081
OW = W - KW + 1  # 4081
NCORES = 8

A = 16  # output block rows (= strip height)
B = 8  # output block cols
NB = W // B  # 512 bases per strip row
TW = NB + 4  # tile width, padded for the v-shift (n+v <= 513)
SPT = 32  # strips per core (32*16 = 512 output rows/core)
NS = 256  # total strips (covers output rows 0..4095)
NU, NV = 2, 3  # row / col pass counts

DT = mybir.dt.bfloat16
NP_BF16 = mybir.dt.np(mybir.dt.bfloat16)

_build_cache = {}


def _build():
    loop = int(os.environ.get("CONV_LOOP", "1"))
    splitq = int(os.environ.get("CONV_SPLITQ", "0"))
    psum_bufs = int(os.environ.get("CONV_PSUM", "4"))
    outq = os.environ.get("CONV_OUTQ", "gpsimd")
    key = (loop, splitq, psum_bufs, outq)
    if key in _build_cache:
        return _build_cache[key]

    nc = bacc.Bacc()
    xt_d = nc.dram_tensor("xt", [SPT + 1, 128, TW], DT, kind="ExternalInput")
    wq_d = nc.dram_tensor("wq", [128, NU * NV, 128], DT, kind="ExternalInput")
    bias_d = nc.dram_tensor("biasb", [128, 1], mybir.dt.float32, kind="ExternalInput")
    out_d = nc.dram_tensor("outp", [SPT, 128, NB], DT, kind="ExternalOutput")

    with tile.TileContext(nc) as tc:
        with (
            tc.tile_pool(name="const", bufs=1) as cpool,
            tc.tile_pool(name="xtiles", bufs=SPT + 2) as spool,
            tc.tile_pool(name="obuf", bufs=4) as opool,
            tc.tile_pool(name="psum", bufs=psum_bufs, space="PSUM") as ppool,
            tc.tile_pool(name="wpsum", bufs=1, space="PSUM") as wpool,
        ):
            wq = cpool.tile([128, NU * NV, 128], DT)
            nc.scalar.dma_start(wq[:], wq_d[:])
            bias_sb = cpool.tile([128, 1], mybir.dt.float32)
            nc.scalar.dma_start(bias_sb[:], bias_d[:])

            # HAM warm-up: the PE clock sits at 4/8 until it has been busy
            # for one ~3.4us activity window. The real chains only start
            # once the first two strip tiles land (~2.5us of DMA head), so
            # without this the first ~25 real matmuls run at half clock.
            # Stream small dummy matmuls on a memset tile while the DMAs
            # are in flight; never read the result.
            warm = cpool.tile([128, 32], DT)
            nc.vector.memset(warm[:], 0)
            wps = wpool.tile([32, 32], mybir.dt.float32, name="warmps")
            for _ in range(40):
                nc.tensor.matmul(
                    wps[:, :], warm[:, 0:32], warm[:, 0:32],
                    start=True, stop=True,
                )

            def body():
                tiles = {}

                def get_tile(t):
                    if t not in tiles:
                        tt = spool.tile([128, TW], DT, tag="xt", name=f"xt{t}")
                        if splitq:
                            eng = nc.sync if t % 2 == 0 else nc.scalar
                        else:
                            eng = nc.sync
                        eng.dma_start(tt[:], xt_d[t])
                        tiles[t] = tt
                    return tiles[t]

                for s in range(SPT):
                    t0 = get_tile(s)
                    t1 = get_tile(s + 1)
                    ps = ppool.tile(
                        [128, NB], mybir.dt.float32, tag="ps", name=f"ps{s}"
                    )
                    k = 0
                    for tt in (t0, t1):
                        for v in range(NV):
                            nc.tensor.matmul(
                                ps[:, :],
                                wq[:, k, :],
                                tt[:, v : v + NB],
                                start=(k == 0),
                                stop=(k == NU * NV - 1),
                            )
                            k += 1
                    ot = opool.tile([128, NB], DT, tag="ot", name=f"ot{s}")
                    nc.vector.tensor_scalar_add(ot[:, :], ps[:, :], bias_sb[:])
                    oeng = {"gpsimd": nc.gpsimd, "sync": nc.sync,
                            "scalar": nc.scalar}[outq]
                    oeng.dma_start(out_d[s], ot[:, :])

            if loop > 1:
                with tc.For_i(0, loop, 1):
                    body()
            else:
                body()
    nc.finalize()
    _build_cache[key] = nc
    return nc


def _host_prep(X, weight, bias):
    Xb = np.ascontiguousarray(X.astype(NP_BF16))
    # Pad rows to (NS+1)*A and cols to TW*B with zeros, then reshape so
    # Xr[t, rho*8+gamma, n] = Xpad[A*t + rho, B*n + gamma].
    Xp = np.zeros(((NS + 1) * A, TW * B), dtype=NP_BF16)
    Xp[:H, :W] = Xb
    Xr = np.ascontiguousarray(
        Xp.reshape(NS + 1, A, TW, B).transpose(0, 1, 3, 2).reshape(NS + 1, 128, TW)
    )

    wb = weight.astype(NP_BF16)
    # wq[rho*8+gamma, u*NV+v, a*8+b] = W[rho-a+16u, gamma-b+8v] where valid
    wq = np.zeros((128, NU * NV, 128), dtype=NP_BF16)
    rho = np.arange(A)[:, None, None, None]  # [A,1,1,1]
    gam = np.arange(B)[None, :, None, None]  # [1,B,1,1]
    aa = np.arange(A)[None, None, :, None]  # [1,1,A,1]
    bb = np.arange(B)[None, None, None, :]  # [1,1,1,B]
    for u in range(NU):
        for v in range(NV):
            r = rho - aa + 16 * u  # [A,1,A,1]
            c = gam - bb + 8 * v  # [1,B,1,B]
            valid = (0 <= r) & (r < KH) & (0 <= c) & (c < KW)
            vals = wb[np.clip(r, 0, KH - 1), np.clip(c, 0, KW - 1)]
            vals = np.where(valid, vals, np.zeros((), dtype=NP_BF16))
            wq[:, u * NV + v, :] = vals.reshape(128, 128)
    biasb = np.full((128, 1), np.float32(bias[0]), dtype=np.float32)

    in_maps = []
    for c in range(NCORES):
        xt = np.ascontiguousarray(Xr[SPT * c : SPT * c + SPT + 1])
        in_maps.append({"xt": xt, "wq": wq, "biasb": biasb})
    return in_maps


def _host_post(results):
    rows = []
    for c in range(NCORES):
        outp = np.asarray(results[c]["outp"])  # [SPT, 128, NB] bf16
        blk = (
            outp.reshape(SPT, A, B, NB)
            .transpose(0, 1, 3, 2)
            .reshape(SPT * A, W)
        )
        rows.append(blk)
    full = np.concatenate(rows, axis=0)  # [4096, 4096]
    return np.ascontiguousarray(full[:OH, :OW]).astype(np.float32)


def kernel(X, weight, bias):
    X = np.asarray(X, dtype=np.float32)
    weight = np.asarray(weight, dtype=np.float32)
    bias = np.asarray(bias, dtype=np.float32)
    nc = _build()
    in_maps = _host_prep(X, weight, bias)
    res = run_bass_kernel_spmd(nc, in_maps, core_ids=list(range(NCORES)))
    return _host_post(res.results)


def _run(X, weight, bias, dt_name=None, trace=False):
    """Compatibility entry for test.py: returns (output, results)."""
    X = np.asarray(X, dtype=np.float32)
    weight = np.asarray(weight, dtype=np.float32)
    bias = np.asarray(bias, dtype=np.float32)
    nc = _build()
    in_maps = _host_prep(X, weight, bias)
    res = run_bass_kernel_spmd(
        nc, in_maps, core_ids=list(range(NCORES)), trace=trace
    )
    return _host_post(res.results), res


# revision 10
# speedup vs baseline: 1.1436x; 1.1436x over previous
"""Trainium2 Bass kernel v2: 4096x4096 fp32 image, 16x16 valid cross-corr + bias.

Block-output scheme: each psum element holds one output of a 16x8 block,
  psum[m=(a,b), n] = out[16s + a, 8n + b],
and the contraction dim holds a 16x8 patch of X,
  D_t[p=(rho,gamma), n] = X[16t + rho, 8n + gamma]
(i.e. X reshaped so partition = (row%16)*8 + col%8, free = col//8).
Each kernel tap (r, c) decomposes uniquely as r = rho - a + 16u (u in {0,1}),
c = gamma - b + 8v (v in {0,1,2}), so SIX matmuls accumulate the full 16x16
conv for 16 output rows x 4096 cols at once:
  psum += S_uv.T @ D_{s+u}[:, v:v+512],
  S_uv[p, m] = W[rho - a + 16u, gamma - b + 8v] (where in range, else 0).
This streams 6x512 PE columns per 16 output rows (vs 10x512 per 17 rows
for the banded-phase scheme) and loads each X byte exactly once (strips
stride = strip height; no halo duplication in HBM traffic).

Per strip: one 6-matmul chain into a single PSUM bank, then one DVE
tensor_scalar_add (+bias, bf16 cast) to SBUF, then SWDGE store. All
storage bf16 (fp32 PSUM accumulation); rel err vs fp32 reference ~3e-3.
Output rows sharded across 8 cores (512 rows each, 32 strips of 16);
weights and bias replicated. A short dummy-matmul stream at kernel
start warms the PE HAM clock gate while the first input DMAs land.

Six passes is optimal for single-level 128-partition block schemes:
passes >= (A+KH-1)(B+KW-1)/128 over output blocks A*B=128, minimized
at (16,8) -> 5.57 -> 6. Measured ~19-25us/core steady state (vs ~63-79
for the banded-phase predecessor), against a ~23us/core HBM floor
(4.3MB in + 4.1MB out at ~358GB/s) and ~25us of PE streaming at the
measured ~131ns per N=512 bf16 matmul.

Env (bench only): CONV_LOOP wraps the body in a hardware For_i loop;
CONV_SPLITQ/CONV_PSUM/CONV_OUTQ are A/B knobs (defaults won).
"""
import os

import numpy as np

import concourse.mybir as mybir
import concourse.tile as tile
from concourse import bacc
from concourse.bass_utils import run_bass_kernel_spmd

H = 4096
W = 4096
KH = 16
KW = 16
OH = H - KH + 1  # 4081
OW = W - KW + 1  # 4081
NCORES = 8

A = 16  # output block rows (= strip height)
B = 8  # output block cols
NB = W // B  # 512 bases per strip row
TW = NB + 4  # tile width, padded for the v-shift (n+v <= 513)
SPT = 32  # strips per core (32*16 = 512 output rows/core)
NS = 256  # total strips (covers output rows 0..4095)
NU, NV = 2, 3  # row / col pass counts

DT = mybir.dt.bfloat16
NP_BF16 = mybir.dt.np(mybir.dt.bfloat16)

_build_cache = {}


def _build():
    loop = int(os.environ.get("CONV_LOOP", "1"))
    splitq = int(os.environ.get("CONV_SPLITQ", "0"))
    psum_bufs = int(os.environ.get("CONV_PSUM", "4"))
    outq = os.environ.get("CONV_OUTQ", "gpsimd")
    key = (loop, splitq, psum_bufs, outq)
    if key in _build_cache:
        return _build_cache[key]

    nc = bacc.Bacc()
    xt_d = nc.dram_tensor("xt", [SPT + 1, 128, TW], DT, kind="ExternalInput")
    wq_d = nc.dram_tensor("wq", [128, NU * NV, 128], DT, kind="ExternalInput")
    bias_d = nc.dram_tensor("biasb", [128, 1], mybir.dt.float32, kind="ExternalInput")
    out_d = nc.dram_tensor("outp", [SPT, 128, NB], DT, kind="ExternalOutput")

    with tile.TileContext(nc) as tc:
        with (
            tc.tile_pool(name="const", bufs=1) as cpool,
            tc.tile_pool(name="xtiles", bufs=SPT + 2) as spool,
            tc.tile_pool(name="obuf", bufs=4) as opool,
            tc.tile_pool(name="psum", bufs=psum_bufs, space="PSUM") as ppool,
            tc.tile_pool(name="wpsum", bufs=1, space="PSUM") as wpool,
        ):
            wq = cpool.tile([128, NU * NV, 128], DT)
            nc.scalar.dma_start(wq[:], wq_d[:])
            bias_sb = cpool.tile([128, 1], mybir.dt.float32)
            nc.scalar.dma_start(bias_sb[:], bias_d[:])

            # HAM warm-up: the PE clock sits at 4/8 until it has been busy
            # for one ~3.4us activity window. The real chains only start
            # once the first two strip tiles land (~2.5us of DMA head), so
            # without this the first ~25 real matmuls run at half clock.
            # Stream small dummy matmuls on a memset tile while the DMAs
            # are in flight; never read the result.
            warm = cpool.tile([128, 32], DT)
            nc.vector.memset(warm[:], 0)
            wps = wpool.tile([32, 32], mybir.dt.float32, name="warmps")
            for _ in range(40):
                nc.tensor.matmul(
                    wps[:, :], warm[:, 0:32], warm[:, 0:32],
                    start=True, stop=True,
                )

            def body():
                tiles = {}

                def get_tile(t):
                    if t not in tiles:
                        tt = spool.tile([128, TW], DT, tag="xt", name=f"xt{t}")
                        if splitq:
                            eng = nc.sync if t % 2 == 0 else nc.scalar
                        else:
                            eng = nc.sync
                        eng.dma_start(tt[:], xt_d[t])
                        tiles[t] = tt
                    return tiles[t]

                for s in range(SPT):
                    t0 = get_tile(s)
                    t1 = get_tile(s + 1)
                    ps = ppool.tile(
                        [128, NB], mybir.dt.float32, tag="ps", name=f"ps{s}"
                    )
                    k = 0
                    for tt in (t0, t1):
                        for v in range(NV):
                            nc.tensor.matmul(
                                ps[:, :],
                                wq[:, k, :],
                                tt[:, v : v + NB],
                                start=(k == 0),
                                stop=(k == NU * NV - 1),
                            )
                            k += 1
                    ot = opool.tile([128, NB], DT, tag="ot", name=f"ot{s}")
                    nc.vector.tensor_scalar_add(ot[:, :], ps[:, :], bias_sb[:])
                    oeng = {"gpsimd": nc.gpsimd, "sync": nc.sync,
                            "scalar": nc.scalar}[outq]
                    oeng.dma_start(out_d[s], ot[:, :])

            if loop > 1:
                with tc.For_i(0, loop, 1):
                    body()
            else:
                body()
    nc.finalize()
    _build_cache[key] = nc
    return nc


def _host_prep(X, weight, bias):
    Xb = np.ascontiguousarray(X.astype(NP_BF16))
    # Pad rows to (NS+1)*A and cols to TW*B with zeros, then reshape so
    # Xr[t, rho*8+gamma, n] = Xpad[A*t + rho, B*n + gamma].
    Xp = np.zeros(((NS + 1) * A, TW * B), dtype=NP_BF16)
    Xp[:H, :W] = Xb
    Xr = np.ascontiguousarray(
        Xp.reshape(NS + 1, A, TW, B).transpose(0, 1, 3, 2).reshape(NS + 1, 128, TW)
    )

    wb = weight.astype(NP_BF16)
    # wq[rho*8+gamma, u*NV+v, a*8+b] = W[rho-a+16u, gamma-b+8v] where valid
    wq = np.zeros((128, NU * NV, 128), dtype=NP_BF16)
    rho = np.arange(A)[:, None, None, None]  # [A,1,1,1]
    gam = np.arange(B)[None, :, None, None]  # [1,B,1,1]
    aa = np.arange(A)[None, None, :, None]  # [1,1,A,1]
    bb = np.arange(B)[None, None, None, :]  # [1,1,1,B]
    for u in range(NU):
        for v in range(NV):
            r = rho - aa + 16 * u  # [A,1,A,1]
            c = gam - bb + 8 * v  # [1,B,1,B]
            valid = (0 <= r) & (r < KH) & (0 <= c) & (c < KW)
            vals = wb[np.clip(r, 0, KH - 1), np.clip(c, 0, KW - 1)]
            vals = np.where(valid, vals, np.zeros((), dtype=NP_BF16))
            wq[:, u * NV + v, :] = vals.reshape(128, 128)
    biasb = np.full((128, 1), np.float32(bias[0]), dtype=np.float32)

    in_maps = []
    for c in range(NCORES):
        xt = np.ascontiguousarray(Xr[SPT * c : SPT * c + SPT + 1])
        in_maps.append({"xt": xt, "wq": wq, "biasb": biasb})
    return in_maps


def _host_post(results):
    rows = []
    for c in range(NCORES):
        outp = np.asarray(results[c]["outp"])  # [SPT, 128, NB] bf16
        blk = (
            outp.reshape(SPT, A, B, NB)
            .transpose(0, 1, 3, 2)
            .reshape(SPT * A, W)
        )
        rows.append(blk)
    full = np.concatenate(rows, axis=0)  # [4096, 4096]
    return np.ascontiguousarray(full[:OH, :OW]).astype(np.float32)


def kernel(X, weight, bias):
    X = np.asarray(X, dtype=np.float32)
    weight = np.asarray(weight, dtype=np.float32)
    bias = np.asarray(bias, dtype=np.float32)
    nc = _build()
    in_maps = _host_prep(X, weight, bias)
    res = run_bass_kernel_spmd(nc, in_maps, core_ids=list(range(NCORES)))
    return _host_post(res.results)


def _run(X, weight, bias, dt_name=None, trace=False):
    """Compatibility entry for test.py: returns (output, results)."""
    X = np.asarray(X, dtype=np.float32)
    weight = np.asarray(weight, dtype=np.float32)
    bias = np.asarray(bias, dtype=np.float32)
    nc = _build()
    in_maps = _host_prep(X, weight, bias)
    res = run_bass_kernel_spmd(
        nc, in_maps, core_ids=list(range(NCORES)), trace=trace
    )
    return _host_post(res.results), res


# revision 11
# speedup vs baseline: 1.1608x; 1.0150x over previous
"""Trainium2 Bass kernel v2: 4096x4096 fp32 image, 16x16 valid cross-corr + bias.

Block-output scheme: each psum element holds one output of a 16x8 block,
  psum[m=(a,b), n] = out[16s + a, 8n + b],
and the contraction dim holds a 16x8 patch of X,
  D_t[p=(rho,gamma), n] = X[16t + rho, 8n + gamma]
(i.e. X reshaped so partition = (row%16)*8 + col%8, free = col//8).
Each kernel tap (r, c) decomposes uniquely as r = rho - a + 16u (u in {0,1}),
c = gamma - b + 8v (v in {0,1,2}), so SIX matmuls accumulate the full 16x16
conv for 16 output rows x 4096 cols at once:
  psum += S_uv.T @ D_{s+u}[:, v:v+512],
  S_uv[p, m] = W[rho - a + 16u, gamma - b + 8v] (where in range, else 0).
This streams 6x512 PE columns per 16 output rows (vs 10x512 per 17 rows
for the banded-phase scheme) and loads each X byte exactly once (strips
stride = strip height; no halo duplication in HBM traffic).

Per strip: one 6-matmul chain into a single PSUM bank, then one DVE
tensor_scalar_add (+bias, bf16 cast) to SBUF, then SWDGE store. All
storage bf16 (fp32 PSUM accumulation); rel err vs fp32 reference ~3e-3.
Output rows sharded across 8 cores (512 rows each, 32 strips of 16);
weights and bias replicated. A short dummy-matmul stream at kernel
start warms the PE HAM clock gate while the first input DMAs land.

Six passes is optimal for single-level 128-partition block schemes:
passes >= (A+KH-1)(B+KW-1)/128 over output blocks A*B=128, minimized
at (16,8) -> 5.57 -> 6. Measured ~19-25us/core steady state (vs ~63-79
for the banded-phase predecessor). Engine-isolation probes: PE-only
~22.7us (118ns per N=512 bf16 matmul, at the streaming limit); all DMA
traffic alone ~7.4us (3x headroom); drain engines <50% duty. PE-bound.

Env (bench only): CONV_LOOP wraps the body in a hardware For_i loop;
CONV_SPLITQ/CONV_PSUM/CONV_OUTQ are A/B knobs (defaults won).
"""
import os

import numpy as np

import concourse.mybir as mybir
import concourse.tile as tile
from concourse import bacc
from concourse.bass_utils import run_bass_kernel_spmd

H = 4096
W = 4096
KH = 16
KW = 16
OH = H - KH + 1  # 4081
OW = W - KW + 1  # 4081
NCORES = 8

A = 16  # output block rows (= strip height)
B = 8  # output block cols
NB = W // B  # 512 bases per strip row
TW = NB + 4  # tile width, padded for the v-shift (n+v <= 513)
SPT = 32  # strips per core (32*16 = 512 output rows/core)
NS = 256  # total strips (covers output rows 0..4095)
NU, NV = 2, 3  # row / col pass counts

DT = mybir.dt.bfloat16
NP_BF16 = mybir.dt.np(mybir.dt.bfloat16)

_build_cache = {}


def _build():
    loop = int(os.environ.get("CONV_LOOP", "1"))
    splitq = int(os.environ.get("CONV_SPLITQ", "0"))
    psum_bufs = int(os.environ.get("CONV_PSUM", "4"))
    outq = os.environ.get("CONV_OUTQ", "gpsimd")
    key = (loop, splitq, psum_bufs, outq)
    if key in _build_cache:
        return _build_cache[key]

    nc = bacc.Bacc()
    xt_d = nc.dram_tensor("xt", [SPT + 1, 128, TW], DT, kind="ExternalInput")
    wq_d = nc.dram_tensor("wq", [128, NU * NV, 128], DT, kind="ExternalInput")
    bias_d = nc.dram_tensor("biasb", [128, 1], mybir.dt.float32, kind="ExternalInput")
    out_d = nc.dram_tensor("outp", [SPT, 128, NB], DT, kind="ExternalOutput")

    with tile.TileContext(nc) as tc:
        with (
            tc.tile_pool(name="const", bufs=1) as cpool,
            tc.tile_pool(name="xtiles", bufs=SPT + 2) as spool,
            tc.tile_pool(name="obuf", bufs=4) as opool,
            tc.tile_pool(name="psum", bufs=psum_bufs, space="PSUM") as ppool,
            tc.tile_pool(name="wpsum", bufs=1, space="PSUM") as wpool,
        ):
            wq = cpool.tile([128, NU * NV, 128], DT)
            nc.scalar.dma_start(wq[:], wq_d[:])
            bias_sb = cpool.tile([128, 1], mybir.dt.float32)
            nc.scalar.dma_start(bias_sb[:], bias_d[:])

            # HAM warm-up: the PE clock sits at 4/8 until it has been busy
            # for one ~3.4us activity window. The real chains only start
            # once the first two strip tiles land (~2.5us of DMA head), so
            # without this the first ~25 real matmuls run at half clock.
            # Stream small dummy matmuls on a memset tile while the DMAs
            # are in flight; never read the result.
            warm = cpool.tile([128, 32], DT)
            nc.vector.memset(warm[:], 0)
            wps = wpool.tile([32, 32], mybir.dt.float32, name="warmps")
            for _ in range(40):
                nc.tensor.matmul(
                    wps[:, :], warm[:, 0:32], warm[:, 0:32],
                    start=True, stop=True,
                )

            def body():
                tiles = {}

                def get_tile(t):
                    if t not in tiles:
                        tt = spool.tile([128, TW], DT, tag="xt", name=f"xt{t}")
                        if splitq:
                            eng = nc.sync if t % 2 == 0 else nc.scalar
                        else:
                            eng = nc.sync
                        eng.dma_start(tt[:], xt_d[t])
                        tiles[t] = tt
                    return tiles[t]

                for s in range(SPT):
                    t0 = get_tile(s)
                    t1 = get_tile(s + 1)
                    ps = ppool.tile(
                        [128, NB], mybir.dt.float32, tag="ps", name=f"ps{s}"
                    )
                    k = 0
                    for tt in (t0, t1):
                        for v in range(NV):
                            nc.tensor.matmul(
                                ps[:, :],
                                wq[:, k, :],
                                tt[:, v : v + NB],
                                start=(k == 0),
                                stop=(k == NU * NV - 1),
                            )
                            k += 1
                    ot = opool.tile([128, NB], DT, tag="ot", name=f"ot{s}")
                    nc.vector.tensor_scalar_add(ot[:, :], ps[:, :], bias_sb[:])
                    oeng = {"gpsimd": nc.gpsimd, "sync": nc.sync,
                            "scalar": nc.scalar}[outq]
                    oeng.dma_start(out_d[s], ot[:, :])

            if loop > 1:
                with tc.For_i(0, loop, 1):
                    body()
            else:
                body()
    nc.finalize()
    _build_cache[key] = nc
    return nc


def _host_prep(X, weight, bias):
    Xb = np.ascontiguousarray(X.astype(NP_BF16))
    # Pad rows to (NS+1)*A and cols to TW*B with zeros, then reshape so
    # Xr[t, rho*8+gamma, n] = Xpad[A*t + rho, B*n + gamma].
    Xp = np.zeros(((NS + 1) * A, TW * B), dtype=NP_BF16)
    Xp[:H, :W] = Xb
    Xr = np.ascontiguousarray(
        Xp.reshape(NS + 1, A, TW, B).transpose(0, 1, 3, 2).reshape(NS + 1, 128, TW)
    )

    wb = weight.astype(NP_BF16)
    # wq[rho*8+gamma, u*NV+v, a*8+b] = W[rho-a+16u, gamma-b+8v] where valid
    wq = np.zeros((128, NU * NV, 128), dtype=NP_BF16)
    rho = np.arange(A)[:, None, None, None]  # [A,1,1,1]
    gam = np.arange(B)[None, :, None, None]  # [1,B,1,1]
    aa = np.arange(A)[None, None, :, None]  # [1,1,A,1]
    bb = np.arange(B)[None, None, None, :]  # [1,1,1,B]
    for u in range(NU):
        for v in range(NV):
            r = rho - aa + 16 * u  # [A,1,A,1]
            c = gam - bb + 8 * v  # [1,B,1,B]
            valid = (0 <= r) & (r < KH) & (0 <= c) & (c < KW)
            vals = wb[np.clip(r, 0, KH - 1), np.clip(c, 0, KW - 1)]
            vals = np.where(valid, vals, np.zeros((), dtype=NP_BF16))
            wq[:, u * NV + v, :] = vals.reshape(128, 128)
    biasb = np.full((128, 1), np.float32(bias[0]), dtype=np.float32)

    in_maps = []
    for c in range(NCORES):
        xt = np.ascontiguousarray(Xr[SPT * c : SPT * c + SPT + 1])
        in_maps.append({"xt": xt, "wq": wq, "biasb": biasb})
    return in_maps


def _host_post(results):
    rows = []
    for c in range(NCORES):
        outp = np.asarray(results[c]["outp"])  # [SPT, 128, NB] bf16
        blk = (
            outp.reshape(SPT, A, B, NB)
            .transpose(0, 1, 3, 2)
            .reshape(SPT * A, W)
        )
        rows.append(blk)
    full = np.concatenate(rows, axis=0)  # [4096, 4096]
    return np.ascontiguousarray(full[:OH, :OW]).astype(np.float32)


def kernel(X, weight, bias):
    X = np.asarray(X, dtype=np.float32)
    weight = np.asarray(weight, dtype=np.float32)
    bias = np.asarray(bias, dtype=np.float32)
    nc = _build()
    in_maps = _host_prep(X, weight, bias)
    res = run_bass_kernel_spmd(nc, in_maps, core_ids=list(range(NCORES)))
    return _host_post(res.results)


def _run(X, weight, bias, dt_name=None, trace=False):
    """Compatibility entry for test.py: returns (output, results)."""
    X = np.asarray(X, dtype=np.float32)
    weight = np.asarray(weight, dtype=np.float32)
    bias = np.asarray(bias, dtype=np.float32)
    nc = _build()
    in_maps = _host_prep(X, weight, bias)
    res = run_bass_kernel_spmd(
        nc, in_maps, core_ids=list(range(NCORES)), trace=trace
    )
    return _host_post(res.results), res
